# revision 33
# baseline (speedup 1.0000x reference)
"""BiLevelRoutingAttention Trainium2 kernel.

Sharding: data-parallel over (T*B)=8 cores; core = b*4 + t.
Host: windowize + transpose + region-routing top-k (0.005% of FLOPs).
Device pipeline (per core, 64 windows x 128 tokens):
  Stage 1: qkv projection in fp32. k/v spike bits token-major [s, d];
    q spike bits channel-major [d, s] (computed via W-stationary
    matmuls, so no per-window transposes are needed later).
  Stage 2a: per-target gathered kv = sum_j k_j^T [v_j | 1] (bf16,
    exact integer arithmetic), masked into block-diagonal compact
    kvs tiles (f16) for all 64 windows.
  Stage 2b: transposed linear attention per window:
    atT[e,s] = kvs^T qT (per-head blocks), Dt[h,s] via ksum columns,
    normalization with a matmul-broadcast reciprocal, projection
    outT[c,s] = wp^T atT_norm, bias fused into the ACT psum drain.
Top-k indices (depend only on b) are baked in; cores pick their
variant via tc.If(partition_id).  Output is returned channel-major
[C, NTOK] and un-transposed on host.
"""

import numpy as np

# problem constants (hardcoded per contract)
T, B, Lt, Lh, Lw, C = 4, 2, 8, 32, 32, 256
WT, WH, WW = 4, 4, 4
NW = WT * WH * WW              # 64 windows
PT, PH, PW = Lt // WT, Lh // WH, Lw // WW
WS = PT * PH * PW              # 128 tokens per window
H, HD = 8, C // 8
TOPK = 4
NTOK = NW * WS                 # 8192 tokens per (t,b) shard
N_CORES = 8
NBLK = 16                      # 4-window blocks in stage 1

last_results = None            # stashed BassKernelResults for test harness
last_nc = None
last_in_maps = None


def _windowize(x):
    xw = x.reshape(T, B, WT, PT, WH, PH, WW, PW, C)
    xw = xw.transpose(0, 1, 2, 4, 6, 3, 5, 7, 8).reshape(T, B, NW, WS, C)
    return xw


def _unwindowize(ow):
    o = ow.reshape(T, B, WT, WH, WW, PT, PH, PW, C)
    o = o.transpose(0, 1, 2, 5, 3, 6, 4, 7, 8).reshape(T, B, Lt, Lh, Lw, C)
    return o


def _routing_idx(xw32):
    """Mimic reference routing in fp32: region scores -> top-4 window idx."""
    region = xw32.sum(0).mean(2)                           # [B,NW,C]
    scores = np.einsum('bic,bjc->bij', region, region) * np.float32(HD ** -0.5)
    idx = np.argsort(-scores, axis=-1, kind='stable')[:, :, :TOPK]
    return idx                                             # [B,NW,TOPK]


def _build_program(idx_by_b, debug=False):
    import concourse.bass as bass
    import concourse.mybir as mybir
    import concourse.tile as tile
    from concourse import bacc

    f32 = mybir.dt.float32
    f16 = mybir.dt.float16
    bf16 = mybir.dt.bfloat16

    nc = bacc.Bacc("TRN2", target_bir_lowering=False, debug=False,
                   num_devices=N_CORES)

    xwT_h = nc.dram_tensor("xwT_h", [C, NTOK], f16, kind="ExternalInput").ap()
    xwT_l = nc.dram_tensor("xwT_l", [C, NTOK], f16, kind="ExternalInput").ap()
    wq_h = nc.dram_tensor("wq_h", [C, 3 * C], f16, kind="ExternalInput").ap()
    wq_l = nc.dram_tensor("wq_l", [C, 3 * C], f16, kind="ExternalInput").ap()
    bq = nc.dram_tensor("bq", [3 * C], f32, kind="ExternalInput").ap()
    wp = nc.dram_tensor("wp", [C, C], f32, kind="ExternalInput").ap()
    wp_b = nc.dram_tensor("wp_b", [C, C], bf16, kind="ExternalInput").ap()
    bp = nc.dram_tensor("bp", [C], f32, kind="ExternalInput").ap()
    masks = nc.dram_tensor("masks", [128, 528], f32, kind="ExternalInput").ap()
    inds = nc.dram_tensor("inds", [128, 256], bf16, kind="ExternalInput").ap()
    out_d = nc.dram_tensor("out", [C, NTOK], f32, kind="ExternalOutput").ap()

    with tile.TileContext(nc) as tc:
        with (
            tc.tile_pool(name="const", bufs=1) as const_pool,
            tc.tile_pool(name="bits", bufs=1) as bits_pool,
            tc.tile_pool(name="work", bufs=2) as work_pool,
        ):
            # ---- resident constants ----
            wqh_sb = const_pool.tile([128, 2 * 768], f16, tag="wqh")
            wql_sb = const_pool.tile([128, 2 * 768], f16, tag="wql")
            for kc in range(2):
                nc.sync.dma_start(wqh_sb[:, kc * 768:(kc + 1) * 768],
                                  wq_h[kc * 128:(kc + 1) * 128, :])
                nc.sync.dma_start(wql_sb[:, kc * 768:(kc + 1) * 768],
                                  wq_l[kc * 128:(kc + 1) * 128, :])
            wp_sb = const_pool.tile([128, 2 * 256], bf16, tag="wp")
            for kc in range(2):
                nc.sync.dma_start(wp_sb[:, kc * 256:(kc + 1) * 256],
                                  wp_b[kc * 128:(kc + 1) * 128, :])

            ones_row = const_pool.tile([1, 512], f32, tag="ones")
            nc.vector.memset(ones_row, 1.0)
            bq_row = const_pool.tile([1, 768], f32, tag="bqr")
            nc.sync.dma_start(bq_row, bq[None, :])
            mask_sb = const_pool.tile([128, 528], f32, tag="masks")
            nc.sync.dma_start(mask_sb, masks)
            ind_sb = const_pool.tile([128, 256], bf16, tag="ind")
            nc.sync.dma_start(ind_sb, inds)
            from concourse.masks import make_identity
            ident_b = const_pool.tile([128, 128], bf16, tag="idb")
            make_identity(nc, ident_b)

            # channel-major thresholds for q: thr_q[d] = 2 - bq[d]
            bq_col = const_pool.tile([128, 2], f32, tag="bqc")
            nc.sync.dma_start(bq_col[:, 0:1], bq[0:128][:, None])
            nc.sync.dma_start(bq_col[:, 1:2], bq[128:256][:, None])
            thr_q = const_pool.tile([128, 2], f32, tag="thrq")
            nc.vector.tensor_scalar(out=thr_q, in0=bq_col,
                                    scalar1=-1.0, scalar2=2.0,
                                    op0=mybir.AluOpType.mult,
                                    op1=mybir.AluOpType.add)
            # token-major thresholds for k,v: thr[s, o] = 2 - bq[256+o]
            thr_kv = const_pool.tile([128, 512], f32, tag="thr")
            bp_col = const_pool.tile([128, 2], f32, tag="bpc")
            nc.sync.dma_start(bp_col[:, 0:1], bp[0:128][:, None])
            nc.sync.dma_start(bp_col[:, 1:2], bp[128:256][:, None])

            # ---- bit arenas (resident) ----
            k_bits = bits_pool.tile([128, NW * 256], bf16, tag="kb")
            v_ext = bits_pool.tile([128, NW * 257], bf16, tag="vb")
            v_r = v_ext.rearrange("p (w d) -> p w d", d=257)
            nc.vector.memset(v_r[:, :, 256], 1.0)
            qT_bits = bits_pool.tile([128, 2 * NTOK], f16, tag="qtb")
            kvs_arena = bits_pool.tile([128, NW * 528], f16, tag="kvs")

            # ---- stage 1 + 2a: qkv projection + LIF bits, with each
            # target's gathered-kv matmuls emitted as soon as all its
            # routed source windows' bits exist (keeps the PE dense) ----
            def kv_window(n, idx, kv_psum):
                kv0 = kv_psum.tile([128, 257], f32, tag="kv")
                kv1 = kv_psum.tile([128, 257], f32, tag="kv")
                js = [int(j) for j in idx[n]]
                for jj, j in enumerate(js):
                    st, sp = jj == 0, jj == 3
                    nc.tensor.matmul(
                        kv0, k_bits[:, j * 256:j * 256 + 128],
                        v_ext[:, j * 257:(j + 1) * 257],
                        start=st, stop=sp)
                    nc.tensor.matmul(
                        kv1, k_bits[:, j * 256 + 128:(j + 1) * 256],
                        v_ext[:, j * 257:(j + 1) * 257],
                        start=st, stop=sp)
                kvs = kvs_arena[:, n * 528:(n + 1) * 528]
                for hf, kvh in enumerate([kv0, kv1]):
                    nc.vector.tensor_tensor(
                        out=kvs[:, hf * 264:hf * 264 + 256],
                        in0=kvh[:, 0:256],
                        in1=mask_sb[:, hf * 264:hf * 264 + 256],
                        op=mybir.AluOpType.mult)
                    nc.vector.tensor_tensor(
                        out=kvs[:, hf * 264 + 256:hf * 264 + 264],
                        in0=kvh[:, 256:257].to_broadcast([128, 8]),
                        in1=mask_sb[:, hf * 264 + 256:hf * 264 + 264],
                        op=mybir.AluOpType.mult)

            emitted = [set(), set()]
            pid = nc.partition_id()
            with (
                tc.tile_pool(name="xt", bufs=2) as xt_pool,
                tc.tile_pool(name="s1_ps", bufs=2, space="PSUM") as s1_psum,
                tc.tile_pool(name="q_ps", bufs=2, space="PSUM") as q_psum,
                tc.tile_pool(name="kv_ps", bufs=4, space="PSUM") as kv_psum,
            ):
                # broadcast 2 - b_qkv[256:768] across partitions
                bc_ps = s1_psum.tile([128, 512], f32, tag="kqv")
                nc.tensor.matmul(bc_ps, ones_row[:, 0:128],
                                 bq_row[:, 256:768], start=True, stop=True)
                nc.vector.tensor_scalar(out=thr_kv, in0=bc_ps,
                                        scalar1=-1.0, scalar2=2.0,
                                        op0=mybir.AluOpType.mult,
                                        op1=mybir.AluOpType.add)
                for blk in range(NBLK):
                    xt0h = xt_pool.tile([128, 512], f16, tag="xt0h")
                    xt0l = xt_pool.tile([128, 512], f16, tag="xt0l")
                    xt1h = xt_pool.tile([128, 512], f16, tag="xt1h")
                    xt1l = xt_pool.tile([128, 512], f16, tag="xt1l")
                    sl = slice(blk * 512, (blk + 1) * 512)
                    nc.sync.dma_start(xt0h, xwT_h[0:128, sl])
                    nc.sync.dma_start(xt0l, xwT_l[0:128, sl])
                    nc.sync.dma_start(xt1h, xwT_h[128:256, sl])
                    nc.sync.dma_start(xt1l, xwT_l[128:256, sl])
                    # q channel-major: qT[d, s] for the whole block
                    for dch in range(2):
                        qps = q_psum.tile([128, 512], f32, tag="qT")
                        dsl0 = slice(dch * 128, (dch + 1) * 128)
                        dsl1 = slice(768 + dch * 128, 768 + (dch + 1) * 128)
                        nc.tensor.matmul(qps, wqh_sb[:, dsl0], xt0h,
                                         start=True, stop=False)
                        nc.tensor.matmul(qps, wql_sb[:, dsl0], xt0h,
                                         start=False, stop=False)
                        nc.tensor.matmul(qps, wqh_sb[:, dsl0], xt0l,
                                         start=False, stop=False)
                        nc.tensor.matmul(qps, wqh_sb[:, dsl1], xt1h,
                                         start=False, stop=False)
                        nc.tensor.matmul(qps, wql_sb[:, dsl1], xt1h,
                                         start=False, stop=False)
                        nc.tensor.matmul(qps, wqh_sb[:, dsl1], xt1l,
                                         start=False, stop=True)
                        nc.vector.tensor_tensor(
                            out=qT_bits[:, dch * NTOK + blk * 512:
                                        dch * NTOK + (blk + 1) * 512],
                            in0=qps,
                            in1=thr_q[:, dch:dch + 1].to_broadcast([128, 512]),
                            op=mybir.AluOpType.is_ge)
                    # k,v token-major per window
                    for w in range(4):
                        n = blk * 4 + w
                        wsl = slice(w * 128, (w + 1) * 128)
                        ps = s1_psum.tile([128, 512], f32, tag="kqv")
                        nc.tensor.matmul(ps, xt0h[:, wsl], wqh_sb[:, 256:768],
                                         start=True, stop=False)
                        nc.tensor.matmul(ps, xt0h[:, wsl], wql_sb[:, 256:768],
                                         start=False, stop=False)
                        nc.tensor.matmul(ps, xt0l[:, wsl], wqh_sb[:, 256:768],
                                         start=False, stop=False)
                        nc.tensor.matmul(ps, xt1h[:, wsl], wqh_sb[:, 1024:1536],
                                         start=False, stop=False)
                        nc.tensor.matmul(ps, xt1h[:, wsl], wql_sb[:, 1024:1536],
                                         start=False, stop=False)
                        nc.tensor.matmul(ps, xt1l[:, wsl], wqh_sb[:, 1024:1536],
                                         start=False, stop=True)
                        nc.vector.tensor_tensor(
                            out=k_bits[:, n * 256:(n + 1) * 256],
                            in0=ps[:, 0:256], in1=thr_kv[:, 0:256],
                            op=mybir.AluOpType.is_ge)
                        nc.vector.tensor_tensor(
                            out=v_r[:, n, 0:256],
                            in0=ps[:, 256:512], in1=thr_kv[:, 256:512],
                            op=mybir.AluOpType.is_ge)
                    avail = (blk + 1) * 4
                    ready = [[], []]
                    for bb in range(2):
                        for n in range(NW):
                            if n in emitted[bb]:
                                continue
                            if max(int(j) for j in idx_by_b[bb][n]) < avail:
                                ready[bb].append(n)
                                emitted[bb].add(n)
                    if ready[0] or ready[1]:
                        with tc.If(pid <= 3) as cmp:
                            for n in ready[0]:
                                kv_window(n, idx_by_b[0], kv_psum)
                        with cmp.Else():
                            for n in ready[1]:
                                kv_window(n, idx_by_b[1], kv_psum)

            # ---- stage 2b: transposed attention + projection ----
            # 4 windows per iteration; per-window reciprocals land in
            # 32-aligned column blocks so one PE transpose serves all 4.
            def attn_stage():
                with (
                    tc.tile_pool(name="at_ps", bufs=1, space="PSUM") as at_psum,
                    tc.tile_pool(name="sm_ps", bufs=1, space="PSUM") as sm_psum,
                    tc.tile_pool(name="pj_ps", bufs=1, space="PSUM") as pj_psum,
                ):
                    for q4 in range(NW // 4):
                        ns = [q4 * 4 + w for w in range(4)]
                        # layout: at_ps/rd_ps cols = chunk*512 + w*128
                        at_ps = at_psum.tile([128, 1024], f32, tag="at")
                        dt_ps = sm_psum.tile([128, 128], f32, tag="dt")
                        for w, n in enumerate(ns):
                            kvs = kvs_arena[:, n * 528:(n + 1) * 528]
                            qT0 = qT_bits[:, n * 128:(n + 1) * 128]
                            qT1 = qT_bits[:, NTOK + n * 128:NTOK + (n + 1) * 128]
                            nc.tensor.matmul(at_ps[:, w * 128:(w + 1) * 128],
                                             kvs[:, 0:128], qT0,
                                             start=True, stop=True)
                            nc.tensor.matmul(at_ps[:, 512 + w * 128:512 + (w + 1) * 128],
                                             kvs[:, 264 + 128:264 + 256],
                                             qT1, start=True, stop=True)
                            nc.tensor.matmul(dt_ps[:, w * 32:w * 32 + 4],
                                             qT0, kvs[:, 256:260],
                                             start=True, stop=True)
                            nc.tensor.matmul(dt_ps[:, w * 32 + 4:w * 32 + 8],
                                             qT1, kvs[:, 264 + 260:264 + 264],
                                             start=True, stop=True)
                        rd = work_pool.tile([128, 128], bf16, tag="rd")
                        # one full-bank read: orders the DVE after ALL four
                        # windows' Dt matmuls (partial reads of a bank the
                        # PE is still writing are a fatal PSUM collision)
                        dsum = work_pool.tile([128, 128], f32, tag="dsum")
                        nc.vector.tensor_scalar_add(dsum, dt_ps, 1e-6)
                        with nc.allow_low_precision(
                                reason="1/D to bf16: 2^-9 rel, tol 2e-2"):
                            for w in range(4):
                                nc.vector.reciprocal(
                                    rd[:, w * 32:w * 32 + 8],
                                    dsum[:, w * 32:w * 32 + 8])
                        rdT_ps = sm_psum.tile([32, 512], bf16, tag="rdt")
                        for w in range(4):
                            nc.tensor.transpose(
                                rdT_ps[:, w * 128:(w + 1) * 128],
                                rd[:, w * 32:(w + 1) * 32], ident_b)
                        rdT = work_pool.tile([32, 512], bf16, tag="rdts")
                        nc.scalar.copy(rdT, rdT_ps)
                        rd_ps = sm_psum.tile([128, 1024], f32, tag="rdbc")
                        for w in range(4):
                            rr = rdT[0:8, w * 128:(w + 1) * 128]
                            ii = ind_sb[0:8, :]
                            nc.tensor.matmul(rd_ps[:, w * 128:(w + 1) * 128],
                                             ii[:, 0:128], rr,
                                             start=True, stop=True)
                            nc.tensor.matmul(rd_ps[:, 512 + w * 128:512 + (w + 1) * 128],
                                             ii[:, 128:256], rr,
                                             start=True, stop=True)
                        rdbc = work_pool.tile([128, 1024], bf16, tag="rdbc")
                        nc.scalar.copy(rdbc, rd_ps)
                        at_n = work_pool.tile([128, 1024], bf16, tag="atn")
                        nc.vector.tensor_tensor(out=at_n[:, 0:512],
                                                in0=at_ps[:, 0:512],
                                                in1=rdbc[:, 0:512],
                                                op=mybir.AluOpType.mult)
                        nc.vector.tensor_tensor(out=at_n[:, 512:1024],
                                                in0=at_ps[:, 512:1024],
                                                in1=rdbc[:, 512:1024],
                                                op=mybir.AluOpType.mult)
                        pj = pj_psum.tile([128, 1024], f32, tag="pj")
                        nc.tensor.matmul(pj[:, 0:512], wp_sb[:, 0:128],
                                         at_n[:, 0:512], start=True, stop=False)
                        nc.tensor.matmul(pj[:, 0:512], wp_sb[:, 256:384],
                                         at_n[:, 512:1024], start=False, stop=True)
                        nc.tensor.matmul(pj[:, 512:1024], wp_sb[:, 128:256],
                                         at_n[:, 0:512], start=True, stop=False)
                        nc.tensor.matmul(pj[:, 512:1024], wp_sb[:, 384:512],
                                         at_n[:, 512:1024], start=False, stop=True)
                        ob = work_pool.tile([128, 1024], f32, tag="ob")
                        nc.scalar.activation(
                            out=ob[:, 0:512], in_=pj[:, 0:512],
                            func=mybir.ActivationFunctionType.Identity,
                            bias=bp_col[:, 0:1], scale=1.0)
                        nc.vector.tensor_scalar_add(
                            ob[:, 512:1024], pj[:, 512:1024],
                            bp_col[:, 1:2])
                        nc.sync.dma_start(
                            out_d[0:128, ns[0] * 128:(ns[0] + 4) * 128],
                            ob[:, 0:512])
                        nc.sync.dma_start(
                            out_d[128:256, ns[0] * 128:(ns[0] + 4) * 128],
                            ob[:, 512:1024])

            attn_stage()

    nc.compile()
    return nc


def _make_masks():
    mask = np.zeros((128, 528), np.float32)
    for hf in range(2):
        for cr in range(128):
            h = hf * 4 + cr // 32                  # global head of row cr
            mask[cr, hf * 264 + h * 32:hf * 264 + (h + 1) * 32] = 1.0
            mask[cr, hf * 264 + 256 + h] = 1.0
    # head -> e' row indicator for the reciprocal broadcast matmuls,
    # replicated at each 32-row window offset of the quad transpose
    ind = np.zeros((128, 256), np.float32)
    for w in range(4):
        for h in range(8):
            ind[w * 32 + h, h * 32:(h + 1) * 32] = 1.0
    return mask, ind


def kernel(x, W_qkv, b_qkv, W_proj, b_proj):
    global last_results, last_nc, last_in_maps
    from concourse import bass_utils

    x = np.asarray(x, dtype=np.float32)
    xw = _windowize(x)                                     # [T,B,NW,WS,C]
    idx = _routing_idx(xw)                                 # [B,NW,TOPK]

    nc = _build_program(idx)
    mask, ind = _make_masks()
    import ml_dtypes
    ind_b = ind.astype(ml_dtypes.bfloat16)
    wp_bf = np.asarray(W_proj, np.float32).astype(ml_dtypes.bfloat16)
    wq32 = np.asarray(W_qkv, np.float32)
    wqh = wq32.astype(np.float16)
    wql = (wq32 - wqh.astype(np.float32)).astype(np.float16)

    in_maps = []
    for core in range(N_CORES):
        b, t = divmod(core, T)
        xwT_c = np.ascontiguousarray(
            xw[t, b].reshape(NTOK, C).T)                   # [C, NTOK]
        xh = xwT_c.astype(np.float16)
        xl = (xwT_c - xh.astype(np.float32)).astype(np.float16)
        in_maps.append({
            "xwT_h": xh,
            "xwT_l": xl,
            "masks": mask,
            "inds": ind_b,
            "wq_h": wqh,
            "wq_l": wql,
            "bq": np.asarray(b_qkv, np.float32),
            "wp": np.asarray(W_proj, np.float32),
            "wp_b": wp_bf,
            "bp": np.asarray(b_proj, np.float32),
        })

    res = bass_utils.run_bass_kernel_spmd(
        nc, in_maps, core_ids=list(range(N_CORES)), trace=False)
    last_results = res
    last_nc, last_in_maps = nc, in_maps

    ow = np.empty((T, B, NW, WS, C), np.float32)
    for core in range(N_CORES):
        b, t = divmod(core, T)
        ow[t, b] = res.results[core]["out"].T.reshape(NW, WS, C)
    return _unwindowize(ow)
